# revision 3
# baseline (speedup 1.0000x reference)
"""Causal depthwise conv (B=8, L=4096, D=1024, K=15) on 8 TRN2 NeuronCores.

Sharding: channels are split across the 8 cores (128 channels each); every
core processes all 8 batch sequences for its channel slice. Inputs are
re-laid-out on the host to [channels, batch, time] so that on-chip tiles have
channels on SBUF partitions and time on the free dimension — tap shifts are
then plain free-dim offsets.

Per-core kernel:
  out[c, b, t] = sum_k w[k, c] * x_pad[c, b, t + k]
  - TensorE: taps 0..p-1 as float32r diagonal-weight matmuls (full column
    rate) accumulating in PSUM.
  - VectorE: taps p..14 as fused per-partition-scalar multiply-add
    (scalar_tensor_tensor) chain seeded from PSUM (the seed op also
    evacuates PSUM).
"""

from contextlib import ExitStack

import numpy as np

import concourse.bacc as bacc
import concourse.tile as tile
from concourse import mybir
from concourse.bass_utils import run_bass_kernel_spmd

F32 = mybir.dt.float32
F32R = mybir.dt.float32r

B = 8
L = 4096
D = 1024
K = 15
NCORES = 8
CPC = D // NCORES  # channels per core = 128
LP = L + K - 1

PE_TAPS = 11
CHUNK = 1024
MM_N = 512  # one PSUM bank (512 fp32)

_compiled_nc = None
_last_in_maps = None


def _build_nc():
    nc = bacc.Bacc(
        "TRN2",
        target_bir_lowering=False,
        debug=False,
        enable_asserts=True,
        num_devices=NCORES,
    )
    x = nc.dram_tensor("x", [CPC, B, LP], F32R, kind="ExternalInput").ap()
    diag = nc.dram_tensor("diag", [K, CPC, CPC], F32R, kind="ExternalInput").ap()
    w = nc.dram_tensor("w", [CPC, 16], F32, kind="ExternalInput").ap()
    out = nc.dram_tensor("out", [CPC, B, L], F32, kind="ExternalOutput").ap()

    n_chunks = L // CHUNK
    qs = CHUNK // MM_N
    dve_taps = list(range(PE_TAPS, K))

    with tile.TileContext(nc) as tc, ExitStack() as ctx:
        const_pool = ctx.enter_context(tc.tile_pool(name="const", bufs=1))
        xp = ctx.enter_context(tc.tile_pool(name="xp", bufs=2))
        op = ctx.enter_context(tc.tile_pool(name="op", bufs=3))
        accp = ctx.enter_context(tc.tile_pool(name="accp", bufs=2))
        psum_bufs = (8 * 512) // CHUNK
        pp = ctx.enter_context(tc.tile_pool(name="pp", bufs=psum_bufs, space="PSUM"))

        dg = const_pool.tile([CPC, K * CPC], F32R, tag="diag")
        nc.sync.dma_start(
            dg[:].rearrange("p (k m) -> p k m", k=K),
            diag.rearrange("k p m -> p k m"),
        )
        wt = const_pool.tile([CPC, 16], F32, tag="w")
        nc.sync.dma_start(wt[:], w[:])

        for b in range(B):
            xt = xp.tile([CPC, LP], F32R, tag="x")
            nc.sync.dma_start(xt[:], x[:, b, :])
            xf = xt[:].bitcast(F32)

            for ci in range(n_chunks):
                t0 = ci * CHUNK
                ps = pp.tile([CPC, CHUNK], F32, tag="ps", name=f"ps_{b}_{ci}")
                for j in range(PE_TAPS):
                    for q in range(qs):
                        nc.tensor.matmul(
                            ps[:, q * MM_N : (q + 1) * MM_N],
                            dg[:, j * CPC : (j + 1) * CPC],
                            xt[:, t0 + j + q * MM_N : t0 + j + (q + 1) * MM_N],
                            start=(j == 0),
                            stop=(j == PE_TAPS - 1),
                        )

                prev = ps[:, 0:CHUNK]
                for i, k in enumerate(dve_taps):
                    last = i == len(dve_taps) - 1
                    if last:
                        dst = op.tile([CPC, CHUNK], F32, tag="osb", name=f"osb_{b}_{ci}")
                    else:
                        dst = accp.tile([CPC, CHUNK], F32, tag="acc", name=f"acc_{b}_{ci}_{i}")
                    nc.vector.scalar_tensor_tensor(
                        dst[:],
                        xf[:, t0 + k : t0 + k + CHUNK],
                        wt[:, k : k + 1],
                        prev,
                        mybir.AluOpType.mult,
                        mybir.AluOpType.add,
                    )
                    prev = dst[:]

                nc.sync.dma_start(out[:, b, t0 : t0 + CHUNK], prev)

    nc.compile()
    return nc


def kernel(x: np.ndarray, weight: np.ndarray) -> np.ndarray:
    """x: [8, 4096, 1024] fp32, weight: [15, 1, 1024] fp32 ->
    [8, 4096, 1024] fp32 causal depthwise conv."""
    global _compiled_nc
    if _compiled_nc is None:
        _compiled_nc = _build_nc()
    nc = _compiled_nc

    x = np.ascontiguousarray(x, dtype=np.float32)
    wk = np.ascontiguousarray(weight, dtype=np.float32).reshape(K, D)

    in_maps = []
    for c in range(NCORES):
        sl = slice(c * CPC, (c + 1) * CPC)
        xpad = np.zeros((CPC, B, LP), dtype=np.float32)
        xpad[:, :, K - 1 :] = x[:, :, sl].transpose(2, 0, 1)
        wc = wk[:, sl]  # [K, CPC]
        diag = np.zeros((K, CPC, CPC), dtype=np.float32)
        didx = np.arange(CPC)
        diag[:, didx, didx] = wc
        wt = np.zeros((CPC, 16), dtype=np.float32)
        wt[:, :K] = wc.T
        in_maps.append({"x": xpad, "diag": diag, "w": wt})

    global _last_in_maps
    _last_in_maps = in_maps
    res = run_bass_kernel_spmd(nc, in_maps, list(range(NCORES)))

    out = np.empty((B, L, D), dtype=np.float32)
    for c in range(NCORES):
        sl = slice(c * CPC, (c + 1) * CPC)
        out[:, :, sl] = res.results[c]["out"].transpose(1, 2, 0)
    return out
